# revision 32
# baseline (speedup 1.0000x reference)
"""Trainium2 Bass kernel: NF4 dequant + linear, hybrid bf16 + fp8-DR.

y = x @ dequant(weight_q, absmax).T + bias

Sharding: column-parallel over out_features across 8 cores (1376 each).
Contraction k split: kt 0-3 (512 cols) as fp8-e4m3 DoubleRow pairs
(2x PE throughput), kt 4-31 in bf16. fp8 scales folded so products are
unscaled (x stored *4, W stored /4) and accumulate into the same PSUM
group as the bf16 matmuls. Host preps, per core:
  xTb   [3584, 8192] bf16 : x rows 512.. transposed
  x8T   [512, 8192]  f8e4 : x rows 0..511 transposed, *4
  wq    [28,128,1376] bf16: dequantized weights kt 4..31
  wq8   [4,128,1376]  f8e4: dequantized weights kt 0..3, /4
Matmuls kt-outer / o-chunk-inner; token loop phased (A/B/C) so early PE
demand tracks the one-time weight DMA stream.
"""

import numpy as np
import ml_dtypes

import concourse.bacc as bacc
import concourse.mybir as mybir
import concourse.tile as tile
from concourse.alu_op_type import AluOpType
from concourse.bass_utils import run_bass_kernel_spmd

DT = mybir.dt

NF4 = np.array([
    -1.0, -0.6961928009986877, -0.5250730514526367, -0.39491748809814453,
    -0.28444138169288635, -0.18477343022823334, -0.09105003625154495, 0.0,
    0.07958029955625534, 0.16093020141124725, 0.24611230194568634,
    0.33791524171829224, 0.44070982933044434, 0.5626170039176941,
    0.7229568362236023, 1.0], dtype=np.float32)

P = 128
IN_F = 4096
OUT_F = 11008
N_CORES = 8
O_LOC = OUT_F // N_CORES          # 1376 out features per core
S_TOT = 4 * 2048                  # 8192 tokens
K8 = 512                          # contraction cols done in fp8 (kt 0-3)
KT8 = K8 // P                     # 4 fp8 kt -> 2 DoubleRow pairs
KTB = (IN_F - K8) // P            # 28 bf16 kt
SP = 256                          # tokens per x macro tile (2 psum tiles)
NSP = S_TOT // SP                 # 32 x macro tiles
O_CHUNKS = [(1024, 352), (0, 512), (512, 512)]
PHASE_SPS = 14                    # x macro tiles in each phased (A/B/C) pass

_CACHE = {}


def _build():
    nc = bacc.Bacc()
    xTb = nc.dram_tensor("xTb", [IN_F - K8, S_TOT], DT.bfloat16,
                         kind="ExternalInput")
    x8T = nc.dram_tensor("x8T", [K8, S_TOT], DT.float8e4,
                         kind="ExternalInput")
    wq_d = nc.dram_tensor("wq", [KTB, P, O_LOC], DT.bfloat16,
                          kind="ExternalInput")
    wq8_d = nc.dram_tensor("wq8", [KT8, P, O_LOC], DT.float8e4,
                           kind="ExternalInput")
    biasb = nc.dram_tensor("biasb", [1, O_LOC], DT.float32,
                           kind="ExternalInput")
    y = nc.dram_tensor("y", [S_TOT, O_LOC], DT.float32, kind="ExternalOutput")

    with tile.TileContext(nc) as tc:
        with (
            tc.tile_pool(name="w", bufs=1) as wpool,
            tc.tile_pool(name="x", bufs=4) as xp,
            tc.tile_pool(name="o", bufs=4) as op,
            tc.tile_pool(name="ps", bufs=8, space="PSUM") as psp,
            tc.tile_pool(name="c", bufs=1) as cst,
        ):
            biasw = cst.tile([P, O_LOC], DT.float32)
            nc.gpsimd.dma_start(out=biasw[:],
                                in_=biasb[0, :].partition_broadcast(P))

            def load_x(sp):
                s0 = sp * SP
                x8b = xp.tile([P, KT8, SP], DT.float8e4, tag="x8b",
                              name="x8b")
                nc.sync.dma_start(
                    out=x8b[:],
                    in_=x8T[:, s0:s0 + SP].rearrange("(k p) s -> p k s", p=P))
                xb = xp.tile([P, KTB, SP], DT.bfloat16, tag="xb", name="xb")
                for g in range(4):
                    nc.sync.dma_start(
                        out=xb[:, g * 7:(g + 1) * 7, :],
                        in_=xTb[g * 7 * P:(g + 1) * 7 * P, s0:s0 + SP]
                            .rearrange("(k p) s -> p k s", p=P))
                return xb, x8b

            # ---- phase-free schedule: 32 full passes, x loaded once each.
            # First PREF x tiles load before the weight stream so the sync
            # queue serves them first.
            PREF = 2
            sched = [(sp, [0, 1, 2]) for sp in range(NSP)]
            xb_pre = [load_x(sched[j][0]) for j in range(PREF)]

            # ---- weight chunks: straight DMA into persistent tiles.
            # kt-major interleaved across chunks = first-use order of the
            # full passes (kt-outer / oi-inner), so pass 1's consumption
            # tracks DMA arrival; each chunk keeps its own queue.
            wt = {}
            w8t = {}
            qs = [nc.scalar, nc.gpsimd, nc.sync]
            for ktp in range(KT8 // 2):
                for oi, (o0, osz) in enumerate(O_CHUNKS):
                    w8 = wpool.tile([P, 2, osz], DT.float8e4,
                                    tag=f"w8_{oi}_{ktp}", name=f"w8_{oi}_{ktp}")
                    qs[oi % 3].dma_start(
                        out=w8[:],
                        in_=wq8_d[ktp * 2:(ktp + 1) * 2, :, o0:o0 + osz]
                            .rearrange("k p o -> p k o"))
                    w8t[(oi, ktp)] = w8
            for kt in range(KTB):
                for oi, (o0, osz) in enumerate(O_CHUNKS):
                    w_t = wpool.tile([P, osz], DT.bfloat16, tag=f"w_{oi}_{kt}",
                                     name=f"w_{oi}_{kt}")
                    qs[oi % 3].dma_start(out=w_t[:],
                                         in_=wq_d[kt, :, o0:o0 + osz])
                    wt[(oi, kt)] = w_t

            def mm_block(sp, xb, x8b, ois):
                for half in range(2):
                    s0 = sp * SP + half * P
                    ps_ts = {oi: psp.tile([P, O_CHUNKS[oi][1]], DT.float32,
                                          tag="ps", name=f"ps_{sp}_{half}_{oi}")
                             for oi in ois}
                    sl = slice(half * P, (half + 1) * P)
                    for ktp in range(KT8 // 2):
                        for oi in ois:
                            nc.tensor.matmul(
                                ps_ts[oi][:],
                                x8b[:, ktp * 2:(ktp + 1) * 2, sl],
                                w8t[(oi, ktp)][:],
                                perf_mode=mybir.MatmulPerfMode.DoubleRow,
                                start=(ktp == 0), stop=False)
                    for kt in range(KTB):
                        for oi in ois:
                            nc.tensor.matmul(ps_ts[oi][:], xb[:, kt, sl],
                                             wt[(oi, kt)][:],
                                             start=False,
                                             stop=(kt == KTB - 1))
                    for oi in ois:
                        o0, osz = O_CHUNKS[oi]
                        out_t = op.tile([P, osz], DT.float32, tag="out",
                                        name="out_t")
                        nc.vector.tensor_tensor(out_t[:], ps_ts[oi][:],
                                                biasw[:, o0:o0 + osz],
                                                AluOpType.add)
                        nc.scalar.dma_start(out=y[s0:s0 + P, o0:o0 + osz],
                                            in_=out_t[:])

            # ---- phased token loop (x loads two iterations ahead) ----
            for idx, (sp, ois) in enumerate(sched):
                xb_cur = xb_pre.pop(0)
                if idx + PREF < len(sched):
                    xb_pre.append(load_x(sched[idx + PREF][0]))
                mm_block(sp, xb_cur[0], xb_cur[1], ois)

    nc.compile()
    return nc


def _get_nc():
    if 'nc' not in _CACHE:
        _CACHE['nc'] = _build()
    return _CACHE['nc']


def make_in_maps(x, weight_q, absmax, bias):
    x = np.asarray(x, dtype=np.float32)
    weight_q = np.asarray(weight_q)
    absmax = np.asarray(absmax, dtype=np.float32)
    bias = np.asarray(bias, dtype=np.float32)
    bf16 = ml_dtypes.bfloat16
    f8 = ml_dtypes.float8_e4m3

    xf = x.reshape(S_TOT, IN_F)
    xTb = np.ascontiguousarray(xf[:, K8:].T.astype(bf16))
    x8T = np.ascontiguousarray((xf[:, :K8].T * 4.0).astype(f8))
    in_maps = []
    for c in range(N_CORES):
        sl = slice(c * O_LOC, (c + 1) * O_LOC)
        q_c = np.ascontiguousarray(weight_q[sl].T)       # [4096, 1376] int32
        am = absmax[sl]                                  # [O_LOC, 64]
        am_exp = am.T.repeat(64, axis=0)                 # [4096, 1376] f32
        w_f32 = NF4[q_c] * am_exp                        # [4096, 1376] f32
        wq = w_f32[K8:].astype(bf16).reshape(KTB, P, O_LOC)
        wq8 = (w_f32[:K8] * 0.25).astype(f8).reshape(KT8, P, O_LOC)
        biasb_c = np.ascontiguousarray(bias[sl].reshape(1, O_LOC))
        in_maps.append({"xTb": xTb, "x8T": x8T,
                        "wq": np.ascontiguousarray(wq),
                        "wq8": np.ascontiguousarray(wq8),
                        "biasb": biasb_c})
    return in_maps


def kernel(x, weight_q, absmax, bias):
    nc = _get_nc()
    in_maps = make_in_maps(x, weight_q, absmax, bias)
    res = run_bass_kernel_spmd(nc, in_maps, core_ids=list(range(N_CORES)))
    y = np.concatenate([res.results[c]["y"] for c in range(N_CORES)], axis=1)
    return np.ascontiguousarray(y.reshape(4, 2048, OUT_F))


# revision 34
# speedup vs baseline: 1.0474x; 1.0474x over previous
"""Trainium2 Bass kernel: NF4 dequant + linear, hybrid bf16 + fp8-DR.

y = x @ dequant(weight_q, absmax).T + bias

Sharding: column-parallel over out_features across 8 cores (1376 each).
Contraction k split: kt 0-3 (512 cols) as fp8-e4m3 DoubleRow pairs
(2x PE throughput), kt 4-31 in bf16. fp8 scales folded so products are
unscaled (x stored *4, W stored /4) and accumulate into the same PSUM
group as the bf16 matmuls. Host preps, per core:
  xTb   [3584, 8192] bf16 : x rows 512.. transposed
  x8T   [512, 8192]  f8e4 : x rows 0..511 transposed, *4
  wq    [28,128,1376] bf16: dequantized weights kt 4..31
  wq8   [4,128,1376]  f8e4: dequantized weights kt 0..3, /4
Matmuls kt-outer / o-chunk-inner; token loop phased (A/B/C) so early PE
demand tracks the one-time weight DMA stream.
"""

import numpy as np
import ml_dtypes

import concourse.bacc as bacc
import concourse.mybir as mybir
import concourse.tile as tile
from concourse.alu_op_type import AluOpType
from concourse.bass_utils import run_bass_kernel_spmd

DT = mybir.dt

NF4 = np.array([
    -1.0, -0.6961928009986877, -0.5250730514526367, -0.39491748809814453,
    -0.28444138169288635, -0.18477343022823334, -0.09105003625154495, 0.0,
    0.07958029955625534, 0.16093020141124725, 0.24611230194568634,
    0.33791524171829224, 0.44070982933044434, 0.5626170039176941,
    0.7229568362236023, 1.0], dtype=np.float32)

P = 128
IN_F = 4096
OUT_F = 11008
N_CORES = 8
O_LOC = OUT_F // N_CORES          # 1376 out features per core
S_TOT = 4 * 2048                  # 8192 tokens
K8 = 512                          # contraction cols done in fp8 (kt 0-3)
KT8 = K8 // P                     # 4 fp8 kt -> 2 DoubleRow pairs
KTB = (IN_F - K8) // P            # 28 bf16 kt
SP = 256                          # tokens per x macro tile (2 psum tiles)
NSP = S_TOT // SP                 # 32 x macro tiles
O_CHUNKS = [(1024, 352), (0, 512), (512, 512)]
PHASE_SPS = 14                    # x macro tiles in each phased (A/B/C) pass

_CACHE = {}


def _build():
    nc = bacc.Bacc()
    xTb = nc.dram_tensor("xTb", [IN_F - K8, S_TOT], DT.bfloat16,
                         kind="ExternalInput")
    x8T = nc.dram_tensor("x8T", [K8, S_TOT], DT.float8e4,
                         kind="ExternalInput")
    wq_d = nc.dram_tensor("wq", [KTB, P, O_LOC], DT.bfloat16,
                          kind="ExternalInput")
    wq8_d = nc.dram_tensor("wq8", [KT8, P, O_LOC], DT.float8e4,
                           kind="ExternalInput")
    biasb = nc.dram_tensor("biasb", [1, O_LOC], DT.float32,
                           kind="ExternalInput")
    y = nc.dram_tensor("y", [S_TOT, O_LOC], DT.float32, kind="ExternalOutput")

    with tile.TileContext(nc) as tc:
        with (
            tc.tile_pool(name="w", bufs=1) as wpool,
            tc.tile_pool(name="x", bufs=4) as xp,
            tc.tile_pool(name="o", bufs=4) as op,
            tc.tile_pool(name="ps", bufs=8, space="PSUM") as psp,
            tc.tile_pool(name="c", bufs=1) as cst,
        ):
            biasw = cst.tile([P, O_LOC], DT.float32)
            nc.gpsimd.dma_start(out=biasw[:],
                                in_=biasb[0, :].partition_broadcast(P))

            def load_x(sp):
                s0 = sp * SP
                x8b = xp.tile([P, KT8, SP], DT.float8e4, tag="x8b",
                              name="x8b")
                nc.sync.dma_start(
                    out=x8b[:],
                    in_=x8T[:, s0:s0 + SP].rearrange("(k p) s -> p k s", p=P))
                xb = xp.tile([P, KTB, SP], DT.bfloat16, tag="xb", name="xb")
                for g in range(4):
                    nc.sync.dma_start(
                        out=xb[:, g * 7:(g + 1) * 7, :],
                        in_=xTb[g * 7 * P:(g + 1) * 7 * P, s0:s0 + SP]
                            .rearrange("(k p) s -> p k s", p=P))
                return xb, x8b

            # ---- phase-free schedule: 32 full passes, x loaded once each.
            # Only sp0's x loads precede the weight stream; sp1's follow it.
            # Weights avoid the sync queue entirely (x has it to itself),
            # so pass 1 has no head-of-line stall behind prefetched x.
            PREF = 2
            sched = [(sp, [0, 1, 2]) for sp in range(NSP)]
            xb_pre = [load_x(sched[0][0])]

            # ---- weight chunks: straight DMA into persistent tiles.
            # kt-major interleaved across chunks = first-use order of the
            # full passes (kt-outer / oi-inner), so pass 1's consumption
            # tracks DMA arrival; chunks 0,2 on scalar, chunk 1 on gpsimd.
            wt = {}
            w8t = {}
            qs = [nc.scalar, nc.gpsimd, nc.scalar]
            for ktp in range(KT8 // 2):
                for oi, (o0, osz) in enumerate(O_CHUNKS):
                    w8 = wpool.tile([P, 2, osz], DT.float8e4,
                                    tag=f"w8_{oi}_{ktp}", name=f"w8_{oi}_{ktp}")
                    qs[oi % 3].dma_start(
                        out=w8[:],
                        in_=wq8_d[ktp * 2:(ktp + 1) * 2, :, o0:o0 + osz]
                            .rearrange("k p o -> p k o"))
                    w8t[(oi, ktp)] = w8
            for kt in range(KTB):
                for oi, (o0, osz) in enumerate(O_CHUNKS):
                    w_t = wpool.tile([P, osz], DT.bfloat16, tag=f"w_{oi}_{kt}",
                                     name=f"w_{oi}_{kt}")
                    qs[oi % 3].dma_start(out=w_t[:],
                                         in_=wq_d[kt, :, o0:o0 + osz])
                    wt[(oi, kt)] = w_t
            xb_pre.append(load_x(sched[1][0]))

            def mm_block(sp, xb, x8b, ois):
                for half in range(2):
                    s0 = sp * SP + half * P
                    ps_ts = {oi: psp.tile([P, O_CHUNKS[oi][1]], DT.float32,
                                          tag="ps", name=f"ps_{sp}_{half}_{oi}")
                             for oi in ois}
                    sl = slice(half * P, (half + 1) * P)
                    for ktp in range(KT8 // 2):
                        for oi in ois:
                            nc.tensor.matmul(
                                ps_ts[oi][:],
                                x8b[:, ktp * 2:(ktp + 1) * 2, sl],
                                w8t[(oi, ktp)][:],
                                perf_mode=mybir.MatmulPerfMode.DoubleRow,
                                start=(ktp == 0), stop=False)
                    for kt in range(KTB):
                        for oi in ois:
                            nc.tensor.matmul(ps_ts[oi][:], xb[:, kt, sl],
                                             wt[(oi, kt)][:],
                                             start=False,
                                             stop=(kt == KTB - 1))
                    for oi in ois:
                        o0, osz = O_CHUNKS[oi]
                        out_t = op.tile([P, osz], DT.float32, tag="out",
                                        name="out_t")
                        nc.vector.tensor_tensor(out_t[:], ps_ts[oi][:],
                                                biasw[:, o0:o0 + osz],
                                                AluOpType.add)
                        nc.scalar.dma_start(out=y[s0:s0 + P, o0:o0 + osz],
                                            in_=out_t[:])

            # ---- phased token loop (x loads two iterations ahead) ----
            for idx, (sp, ois) in enumerate(sched):
                xb_cur = xb_pre.pop(0)
                if idx + PREF < len(sched):
                    xb_pre.append(load_x(sched[idx + PREF][0]))
                mm_block(sp, xb_cur[0], xb_cur[1], ois)

    nc.compile()
    return nc


def _get_nc():
    if 'nc' not in _CACHE:
        _CACHE['nc'] = _build()
    return _CACHE['nc']


def make_in_maps(x, weight_q, absmax, bias):
    x = np.asarray(x, dtype=np.float32)
    weight_q = np.asarray(weight_q)
    absmax = np.asarray(absmax, dtype=np.float32)
    bias = np.asarray(bias, dtype=np.float32)
    bf16 = ml_dtypes.bfloat16
    f8 = ml_dtypes.float8_e4m3

    xf = x.reshape(S_TOT, IN_F)
    xTb = np.ascontiguousarray(xf[:, K8:].T.astype(bf16))
    x8T = np.ascontiguousarray((xf[:, :K8].T * 4.0).astype(f8))
    in_maps = []
    for c in range(N_CORES):
        sl = slice(c * O_LOC, (c + 1) * O_LOC)
        q_c = np.ascontiguousarray(weight_q[sl].T)       # [4096, 1376] int32
        am = absmax[sl]                                  # [O_LOC, 64]
        am_exp = am.T.repeat(64, axis=0)                 # [4096, 1376] f32
        w_f32 = NF4[q_c] * am_exp                        # [4096, 1376] f32
        wq = w_f32[K8:].astype(bf16).reshape(KTB, P, O_LOC)
        wq8 = (w_f32[:K8] * 0.25).astype(f8).reshape(KT8, P, O_LOC)
        biasb_c = np.ascontiguousarray(bias[sl].reshape(1, O_LOC))
        in_maps.append({"xTb": xTb, "x8T": x8T,
                        "wq": np.ascontiguousarray(wq),
                        "wq8": np.ascontiguousarray(wq8),
                        "biasb": biasb_c})
    return in_maps


def kernel(x, weight_q, absmax, bias):
    nc = _get_nc()
    in_maps = make_in_maps(x, weight_q, absmax, bias)
    res = run_bass_kernel_spmd(nc, in_maps, core_ids=list(range(N_CORES)))
    y = np.concatenate([res.results[c]["y"] for c in range(N_CORES)], axis=1)
    return np.ascontiguousarray(y.reshape(4, 2048, OUT_F))
